# revision 16
# baseline (speedup 1.0000x reference)
"""Trainium2 Bass kernel for nn_MultiHeadAttention_867583393876.

Math (per batch b, head h, all matrices 512x512):
  Qm = x[b] @ WQ[h]; Km = x[b] @ WK[h]; Vm = x[b] @ WV[h]
  S  = Qm @ Km                      (the reference's K.reshape is an identity
                                     on a square matrix, so S = Q @ K, not Q@K^T)
  A  = softmax(S / sqrt(512), axis over the QUERY index t (rows of S))
  Zm = A @ Vm
  out[b] = Z.reshape(512, 4096) @ WO   with Z stacked (h, t, e) -> row-scramble:
      out row t' = h*64 + t//8 uses Z_h rows t = 8*(t'%64)+j, j in [0,8).

Head h only feeds output rows [64h, 64h+64), so the kernel is sharded
head-parallel across the 8 cores with NO collectives: core c computes
out[:, 64c:64(c+1), :] for all 16 batches.

Everything is computed in transposed space (partition = channel) so the
softmax reduction runs along the free axis:
  XT = x[b]^T (pre-transposed on host), QmT = WQ^T @ XT, Km/Vm natural,
  ST = Km^T-contract vs QmT, softmax per partition row,
  ZmT = lhsT(Vm natural) @ AT, and the WO stage consumes stride-8 free-dim
  slices of ZmT (which exactly realizes the reference's reshape scramble).

Inputs are declared float32r in DRAM (same 4-byte encoding as fp32) and
DMA'd straight into resident f32r SBUF tiles -- no staging or rounding
copies.  Engine routing is explicit so the PSUM-drain copies and the
softmax chain never queue behind each other:
  Scalar (Act): vm copy, EXP, A-normalize, so copy
  Vector (DVE): qt scale-copy, km copy, zt scatter, MAX reduce, reciprocal
  Sync (SP):    weight/WO/x DMA issue; x prefetch lands during S, not WO
Stage order per batch is Q, K, V, [Z+WO of b-1], S (V after S for the last
batch only, covering its softmax latency before the final Z+WO tail).
"""

import numpy as np

B, T, E, H = 16, 512, 512, 8
N_CORES = 8
SCALE = 1.0 / 22.627416997969522  # 1/sqrt(512)

_CACHE = {}


def _emit(ctx, nc, tc, tile, mybir, aps):
    import concourse.bass as bass

    f32 = mybir.dt.float32
    f32r = mybir.dt.float32r
    bf16 = mybir.dt.bfloat16
    x, wq, wk, wv, wo, outp = (
        aps["x"], aps["wq"], aps["wk"], aps["wv"], aps["wo"], aps["out"],
    )
    ts = bass.ts

    def pool(name, bufs, space="SBUF"):
        return ctx.enter_context(tc.tile_pool(name=name, bufs=bufs, space=space))

    # SBUF pools (KB/partition; 208KB usable)
    p_wo = pool("wo", 1)          # 64KB  WO resident
    p_w = pool("w", 1)            # 24KB  WQ/WK/WV[h] resident
    p_xt = pool("xt", 3)          # 24KB  x[b]^T triple-buffered (depth-2 prefetch)
    p_q = pool("q", 2)            # 16KB  QmT
    p_k = pool("k", 2)            # 16KB  Km natural
    p_v = pool("v", 2)            # 16KB  Vm natural
    p_at = pool("at", 1)          # 8KB   AT
    p_scr = pool("scr", 2)        # 4KB   exp scratch
    p_zt = pool("zt", 1)          # 16KB  ZmT scrambled, batch pair
    p_out = pool("ostage", 1)     # 2KB   output staging
    p_small = pool("small", 4)

    ps_mm = pool("ps_mm", 4, space="PSUM")
    ps_st = pool("ps_st", 3, space="PSUM")
    ps_wo = pool("ps_wo", 1, space="PSUM")

    Copy = mybir.ActivationFunctionType.Copy
    Exp = mybir.ActivationFunctionType.Exp

    # ---- resident weight loads: pure DMA, no copies ----
    # sync-ring order = need order: WQ (interleaved with x0 on the scalar
    # ring), WK, WV, then WO spread over batches 0-1.
    wq_r = p_w.tile([128, 4 * 512], f32r, tag="wq")
    wk_r = p_w.tile([128, 4 * 512], f32r, tag="wk")
    wv_r = p_w.tile([128, 4 * 512], f32r, tag="wv")
    wo_r = p_wo.tile([128, 32 * 512], bf16, tag="wo")
    xload = {}

    def load_x(bb):  # merged DMA on the sync ring
        xt = p_xt.tile([128, 4 * 512], f32r, tag="xt")
        merged_load(nc.sync, xt, x[bb], 4)
        xload[bb] = xt

    # PE p-state warmup: the runtime preamble + first weight DMAs leave the
    # PE idle for ~4us at kernel start, so the first real matmuls would run
    # at the 0.65/1.2GHz ramp states.  A short dummy accumulation keeps the
    # PE streaming (result never read) until the first tiles land.  The
    # warmup tile is a single 128x128 block so its memset clears the vector
    # queue ~2us sooner than a full bank would.
    p_warm = pool("warm", 1)
    wt0 = p_warm.tile([128, 128], f32, tag="warm0")
    nc.vector.memset(wt0[:], 0.0)
    pw = ps_st.tile([128, 512], f32, tag="st", name="warm_ps")
    N_WARM = 9
    for i in range(N_WARM):
        nc.tensor.matmul(
            pw[:, 0:128], wt0[:], wt0[:],
            start=(i == 0), stop=(i == N_WARM - 1),
        )

    def merged_load(eng, dst_tile, dram_ap, ntiles, tile0=0):
        # one DMA for `ntiles` 128x512 tiles: descriptor generation on the
        # issuing ring costs ~700ns per instruction, so per-tile dma_starts
        # serialize the head (~700ns each on the sequencer)
        df = dst_tile[:]
        dst = bass.AP(
            df.tensor, df.offset + tile0 * 512,
            [list(df.ap[0]), [512, ntiles], [1, 512]],
        )
        sf = dram_ap
        srco = sf.offset + tile0 * 65536
        s = bass.AP(sf.tensor, srco, [[512, 128], [65536, ntiles], [1, 512]])
        eng.dma_start(dst, s)

    with nc.named_scope("load_w"):
        # head-critical: per-tile DMAs so tile k lands as early as possible
        # (a merged DMA emits descriptors partition-major, so no tile
        # completes until the whole transfer ends); the rest are merged to
        # save ~700ns of sequencer descriptor-gen per dma_start
        for i in range(4):
            nc.sync.dma_start(wq_r[:, ts(i, 512)], wq[i * 128:(i + 1) * 128, :])
        xt0 = p_xt.tile([128, 4 * 512], f32r, tag="xt", name="xt0")
        for i in range(4):
            nc.scalar.dma_start(xt0[:, ts(i, 512)], x[0, i * 128:(i + 1) * 128, :])
        xload[0] = xt0
        for i in range(4):
            nc.sync.dma_start(wk_r[:, ts(i, 512)], wk[i * 128:(i + 1) * 128, :])
        # wv per-tile too: V(0)'s k-loop consumes tile k at ~18+0.9k us and a
        # merged load completes no tile until the whole 1MB lands
        for i in range(4):
            nc.sync.dma_start(wv_r[:, ts(i, 512)], wv[i * 128:(i + 1) * 128, :])

    zt_state = [None]
    pending = []

    def emit_z(b, vm, at):
        # ---- ZmT = lhsT(Vm natural) @ AT ----
        # PSUM->SBUF drain scatters straight into the WO-ready scrambled
        # layout: ZS free index = vblk*1024 + j*128 + (half*64+q) where the
        # Z column t = 8q + j and half = b%2.
        if b % 2 == 0:
            ztp = p_zt.tile([128, 2 * 4 * 512], bf16, tag="zt")
            zt_state[0] = ztp
        zt = zt_state[0]
        half = b % 2
        for vblk in range(4):
            pz = ps_mm.tile([128, 512], f32, tag="mm")
            for m in range(4):
                nc.tensor.matmul(
                    pz[:],
                    vm[:, m * 512 + vblk * 128: m * 512 + vblk * 128 + 128],
                    at[:, ts(m, 512)],
                    start=(m == 0), stop=(m == 3),
                )
            zf = zt[:]
            pf = pz[:]
            dst = bass.AP(
                zf.tensor, zf.offset + vblk * 1024 + half * 64,
                [list(zf.ap[0]), [1, 64], [128, 8]],
            )
            src = bass.AP(
                pf.tensor, pf.offset, [list(pf.ap[0]), [8, 64], [1, 8]]
            )
            nc.vector.tensor_copy(dst, src)

        # ---- WO stage for the (b-1, b) pair ----
        if b % 2 == 1:
            po = ps_wo.tile([128, 512], f32, tag="wops")
            for kt in range(32):
                # dblk-outer: the vblk-d scatter writes zt block d; reading
                # block 0 first keeps the PE off the blocks still in flight.
                # WO contraction row block for (j, dblk) is tile j*4 + dblk.
                # Exception: the first pair runs while the WO halves are
                # still streaming in tile-order, so keep tile-order there.
                if b == 1:
                    j, dblk = kt // 4, kt % 4
                else:
                    dblk, j = kt // 8, kt % 8
                rhs = wo_r[:, ts(j * 4 + dblk, 512)]
                lhs = zt[:, dblk * 1024 + j * 128: dblk * 1024 + (j + 1) * 128]
                nc.tensor.matmul(
                    po[:], lhs, rhs,
                    start=(kt == 0), stop=(kt == 31),
                )
            so = p_out.tile([128, 512], f32, tag="so")
            nc.scalar.activation(so[:], po[:], Copy)
            # rows of outp[b-1] and outp[b] are contiguous in DRAM: one DMA
            od = bass.AP(
                outp.tensor, outp.offset + (b - 1) * 64 * 512,
                [[512, 128], [1, 512]],
            )
            ring = nc.scalar if b == B - 1 else nc.sync
            ring.dma_start(od, so[:])

    def col(w, k, blk):
        return w[:, k * 512 + blk * 128: k * 512 + blk * 128 + 128]

    def load_wo_chunk(cb):
        # WO tiles are first read at the end of batch 2.  4-tile chunks: the
        # DMA system drains transfers in global issue order, so one 16-tile
        # merged DMA would block the sync sequencer for ~11us of descriptor
        # flow-control and hold every later transfer behind 4MB of FIFO.
        with nc.named_scope(f"load_wo{cb}"):
            merged_load(nc.sync, wo_r, wo, 4, tile0=4 * cb)

    for b in range(B):
        with nc.named_scope(f"batch{b}"):
            pass
            xt = xload.pop(b)

            # ---- QmT = WQ^T @ XT ----
            qt = p_q.tile([128, 4 * 512], f32r, tag="q")
            if b == 0:
                # k-outer: start the PE as soon as wq tile k / x tile k land
                pqs = [
                    ps_mm.tile([128, 512], f32, tag="mm", name=f"pq{i}")
                    for i in range(4)
                ]
                for k in range(4):
                    for dblk in range(4):
                        nc.tensor.matmul(
                            pqs[dblk][:],
                            col(wq_r, k, dblk), xt[:, ts(k, 512)],
                            start=(k == 0), stop=(k == 3),
                        )
                # alternate drain engines: scalar is idle until S(0), and
                # K(0)'s PSUM allocations wait on these (ps_mm recycling)
                for dblk in range(4):
                    if dblk % 2 == 0:
                        nc.vector.tensor_scalar_mul(
                            qt[:, ts(dblk, 512)], pqs[dblk][:], SCALE
                        )
                    else:
                        nc.scalar.activation(
                            qt[:, ts(dblk, 512)], pqs[dblk][:], Copy, scale=SCALE
                        )
            else:
                for dblk in range(4):
                    pq = ps_mm.tile([128, 512], f32, tag="mm")
                    for k in range(4):
                        nc.tensor.matmul(
                            pq[:], col(wq_r, k, dblk), xt[:, ts(k, 512)],
                            start=(k == 0), stop=(k == 3),
                        )
                    # fold the 1/sqrt(512) softmax scale into the drain copy
                    # (vector: the scalar queue still holds b-1's EXP/at ops,
                    # and K's PSUM banks wait on these drains)
                    nc.vector.tensor_scalar_mul(qt[:, ts(dblk, 512)], pq[:], SCALE)

            # ---- Km natural = XT^T-contract @ WK ----
            km = p_k.tile([128, 4 * 512], f32r, tag="k")
            if b == 0:
                pks = [
                    ps_mm.tile([128, 512], f32, tag="mm", name=f"pk{i}")
                    for i in range(4)
                ]
                for k in range(4):
                    for tblk in range(4):
                        nc.tensor.matmul(
                            pks[tblk][:],
                            col(xt, k, tblk), wk_r[:, ts(k, 512)],
                            start=(k == 0), stop=(k == 3),
                        )
                for tblk in range(4):
                    if tblk % 2 == 0:
                        nc.vector.tensor_copy(km[:, ts(tblk, 512)], pks[tblk][:])
                    else:
                        nc.scalar.activation(km[:, ts(tblk, 512)], pks[tblk][:], Copy)
            else:
                for tblk in range(4):
                    pk = ps_mm.tile([128, 512], f32, tag="mm")
                    for k in range(4):
                        nc.tensor.matmul(
                            pk[:], col(xt, k, tblk), wk_r[:, ts(k, 512)],
                            start=(k == 0), stop=(k == 3),
                        )
                    nc.vector.tensor_copy(km[:, ts(tblk, 512)], pk[:])

            # ---- Vm natural = XT^T-contract @ WV ----
            # before Z/S for b<15 so vm's PSUM drains clear the scalar queue
            # early (next batch's Q banks depend on them); for the last batch
            # V runs after S instead, covering the softmax latency before the
            # final Z+WO tail.
            def emit_v():
                vm = p_v.tile([128, 4 * 512], bf16, tag="v", name="vm")
                for tblk in range(4):
                    pv = ps_mm.tile([128, 512], f32, tag="mm", name="pv")
                    for k in range(4):
                        nc.tensor.matmul(
                            pv[:], col(xt, k, tblk), wv_r[:, ts(k, 512)],
                            start=(k == 0), stop=(k == 3),
                        )
                    nc.scalar.activation(vm[:, ts(tblk, 512)], pv[:], Copy)
                return vm

            if b == 0:
                load_x(1)
            vm = None if b == B - 1 else emit_v()
            if b == 0:
                # issue order = need order on the FIFO DMA stream: x2 jumps
                # ahead of WO (it gates batch 2's matmuls at ~50us; WO isn't
                # read before ~65us), then all eight WO chunks.
                load_x(2)
                for cb in range(8):
                    load_wo_chunk(cb)

            # deferred Z + WO of the previous batch: fills the PE while this
            # batch's qt/km drains land, and its matmuls hide S's wait
            if pending:
                emit_z(*pending.pop())

            # prefetch x two batches ahead AFTER the Z+WO stage: the DMA
            # writes then land during S/Q instead of fighting the WO
            # matmuls' zt/wo_r reads for SBUF bandwidth
            if b + 2 < B and b != 0:
                load_x(b + 2)

            # ---- ST = Km^T-contract @ QmT, softmax along free axis ----
            # A and the whole V/Z/WO chain run in bf16: the softmax weights
            # are near-one-hot so bf16 there costs ~0.4% relative error, and
            # bf16 operands halve WO DMA + SBUF read traffic
            at = p_at.tile([128, 4 * 512], bf16, tag="at")
            for sblk in range(4):
                pst = ps_st.tile([128, 512], f32, tag="st")
                for m in range(4):
                    nc.tensor.matmul(
                        pst[:], col(km, m, sblk), qt[:, ts(m, 512)],
                        start=(m == 0), stop=(m == 3),
                    )
                nmx = p_small.tile([128, 1], f32, tag="nmx")
                nc.vector.tensor_reduce(
                    nmx[:], pst[:], axis=mybir.AxisListType.X,
                    op=mybir.AluOpType.max, negate=True,
                )
                scr = p_scr.tile([128, 512], f32, tag="scr")
                sm = p_small.tile([128, 1], f32, tag="sm")
                nc.scalar.activation(
                    scr[:], pst[:], Exp, bias=nmx[:], scale=1.0, accum_out=sm[:],
                )
                rc = p_small.tile([128, 1], f32, tag="rc")
                nc.vector.reciprocal(rc[:], sm[:])
                if b == B - 1:
                    # tail: Z15 needs at+vm; at on vector lets the scalar
                    # queue reach the vm drains sooner
                    nc.vector.tensor_scalar_mul(at[:, ts(sblk, 512)], scr[:], rc[:])
                else:
                    nc.scalar.activation(
                        at[:, ts(sblk, 512)], scr[:], Copy, scale=rc[:]
                    )

            if vm is None:
                vm = emit_v()
            pending.append((b, vm, at))

    emit_z(*pending.pop())


def _build():
    import concourse.bass as bass  # noqa: F401
    import concourse.tile as tile
    from concourse import bacc, mybir

    nc = bacc.Bacc(
        "TRN2",
        target_bir_lowering=False,
        debug=False,
        enable_asserts=False,
        num_devices=N_CORES,
    )
    f32 = mybir.dt.float32
    f32r = mybir.dt.float32r
    aps = {
        "x": nc.dram_tensor("x", (B, E, T), f32r, kind="ExternalInput").ap(),
        "wq": nc.dram_tensor("wq", (E, E), f32r, kind="ExternalInput").ap(),
        "wk": nc.dram_tensor("wk", (E, E), f32r, kind="ExternalInput").ap(),
        "wv": nc.dram_tensor("wv", (E, E), f32r, kind="ExternalInput").ap(),
        "wo": nc.dram_tensor("wo", (H * E, E), mybir.dt.bfloat16, kind="ExternalInput").ap(),
        "out": nc.dram_tensor("out", (B, 64, E), f32, kind="ExternalOutput").ap(),
    }
    from contextlib import ExitStack

    with tile.TileContext(nc) as tc, ExitStack() as ctx:
        _emit(ctx, nc, tc, tile, mybir, aps)
    nc.compile()
    return nc


def _get_nc():
    if "nc" not in _CACHE:
        _CACHE["nc"] = _build()
    return _CACHE["nc"]


def run(inputs, trace=False):
    from concourse.bass_utils import run_bass_kernel_spmd

    nc = _get_nc()
    x = np.asarray(inputs["x"], dtype=np.float32)
    xT = np.ascontiguousarray(x.transpose(0, 2, 1))
    WQ = np.asarray(inputs["WQ"], dtype=np.float32)
    WK = np.asarray(inputs["WK"], dtype=np.float32)
    WV = np.asarray(inputs["WV"], dtype=np.float32)
    import ml_dtypes

    WO = np.ascontiguousarray(
        np.asarray(inputs["WO"], dtype=np.float32).astype(ml_dtypes.bfloat16)
    )
    in_maps = [
        {
            "x": xT,
            "wq": np.ascontiguousarray(WQ[c]),
            "wk": np.ascontiguousarray(WK[c]),
            "wv": np.ascontiguousarray(WV[c]),
            "wo": WO,
        }
        for c in range(N_CORES)
    ]
    res = run_bass_kernel_spmd(
        nc, in_maps, core_ids=list(range(N_CORES)), trace=trace
    )
    out = np.empty((B, T, E), dtype=np.float32)
    for c in range(N_CORES):
        out[:, 64 * c:64 * (c + 1), :] = res.results[c]["out"]
    return out, res


def kernel(**inputs):
    out, _ = run(inputs, trace=False)
    return out



# revision 20
# speedup vs baseline: 1.2070x; 1.2070x over previous
"""Trainium2 Bass kernel for nn_MultiHeadAttention_867583393876.

Math (per batch b, head h, all matrices 512x512):
  Qm = x[b] @ WQ[h]; Km = x[b] @ WK[h]; Vm = x[b] @ WV[h]
  S  = Qm @ Km                      (the reference's K.reshape is an identity
                                     on a square matrix, so S = Q @ K, not Q@K^T)
  A  = softmax(S / sqrt(512), axis over the QUERY index t (rows of S))
  Zm = A @ Vm
  out[b] = Z.reshape(512, 4096) @ WO   with Z stacked (h, t, e) -> row-scramble:
      out row t' = h*64 + t//8 uses Z_h rows t = 8*(t'%64)+j, j in [0,8).

Head h only feeds output rows [64h, 64h+64), so the kernel is sharded
head-parallel across the 8 cores with NO collectives: core c computes
out[:, 64c:64(c+1), :] for all 16 batches.

Everything is computed in transposed space (partition = channel) so the
softmax reduction runs along the free axis:
  XT = x[b]^T (pre-transposed on host), QmT = WQ^T @ XT, Km/Vm natural,
  ST = Km^T-contract vs QmT, softmax per partition row,
  ZmT = lhsT(Vm natural) @ AT, and the WO stage consumes stride-8 free-dim
  slices of ZmT (which exactly realizes the reference's reshape scramble).

Inputs are declared float32r in DRAM (same 4-byte encoding as fp32) and
DMA'd straight into resident f32r SBUF tiles -- no staging or rounding
copies.  Engine routing is explicit so the PSUM-drain copies and the
softmax chain never queue behind each other:
  Scalar (Act): vm copy, EXP, A-normalize, so copy
  Vector (DVE): qt scale-copy, km copy, zt scatter, MAX reduce, reciprocal
  Sync (SP):    weight/WO/x DMA issue; x prefetch lands during S, not WO
Stage order per batch is Q, K, V, [Z+WO of b-1], S (V after S for the last
batch only, covering its softmax latency before the final Z+WO tail).
"""

import numpy as np

B, T, E, H = 16, 512, 512, 8
N_CORES = 8
SCALE = 1.0 / 22.627416997969522  # 1/sqrt(512)

_CACHE = {}


def _emit(ctx, nc, tc, tile, mybir, aps):
    import concourse.bass as bass

    f32 = mybir.dt.float32
    f32r = mybir.dt.float32r
    bf16 = mybir.dt.bfloat16
    x, wq, wk, wv, wo, outp = (
        aps["x"], aps["wq"], aps["wk"], aps["wv"], aps["wo"], aps["out"],
    )
    ts = bass.ts

    def pool(name, bufs, space="SBUF"):
        return ctx.enter_context(tc.tile_pool(name=name, bufs=bufs, space=space))

    # SBUF pools (KB/partition; 208KB usable)
    p_wo = pool("wo", 1)          # 64KB  WO resident
    p_w = pool("w", 1)            # 24KB  WQ/WK/WV[h] resident
    p_xt = pool("xt", 3)          # 24KB  x[b]^T triple-buffered (depth-2 prefetch)
    p_q = pool("q", 2)            # 16KB  QmT
    p_k = pool("k", 2)            # 16KB  Km natural
    p_v = pool("v", 2)            # 16KB  Vm natural
    p_at = pool("at", 1)          # 8KB   AT
    p_scr = pool("scr", 2)        # 4KB   exp scratch
    p_zt = pool("zt", 1)          # 16KB  ZmT scrambled, batch pair
    p_out = pool("ostage", 1)     # 2KB   output staging
    p_small = pool("small", 4)

    ps_mm = pool("ps_mm", 4, space="PSUM")
    ps_st = pool("ps_st", 3, space="PSUM")
    ps_wo = pool("ps_wo", 1, space="PSUM")

    Copy = mybir.ActivationFunctionType.Copy
    Exp = mybir.ActivationFunctionType.Exp

    # ---- resident weight loads: pure DMA, no copies ----
    # sync-ring order = need order: WQ (interleaved with x0 on the scalar
    # ring), WK, WV, then WO spread over batches 0-1.
    wq_r = p_w.tile([128, 4 * 512], f32r, tag="wq")
    wk_r = p_w.tile([128, 4 * 512], f32r, tag="wk")
    wv_r = p_w.tile([128, 4 * 512], f32r, tag="wv")
    wo_r = p_wo.tile([128, 32 * 512], f32r, tag="wo")
    xload = {}

    def load_x(bb):  # merged DMA on the sync ring
        xt = p_xt.tile([128, 4 * 512], f32r, tag="xt")
        merged_load(nc.sync, xt, x[bb], 4)
        xload[bb] = xt

    # PE p-state warmup: the runtime preamble + first weight DMAs leave the
    # PE idle for ~4us at kernel start, so the first real matmuls would run
    # at the 0.65/1.2GHz ramp states.  A short dummy accumulation keeps the
    # PE streaming (result never read) until the first tiles land.  The
    # warmup tile is a single 128x128 block so its memset clears the vector
    # queue ~2us sooner than a full bank would.
    p_warm = pool("warm", 1)
    wt0 = p_warm.tile([128, 128], f32, tag="warm0")
    nc.vector.memset(wt0[:], 0.0)
    pw = ps_st.tile([128, 512], f32, tag="st", name="warm_ps")
    N_WARM = 9
    for i in range(N_WARM):
        nc.tensor.matmul(
            pw[:, 0:128], wt0[:], wt0[:],
            start=(i == 0), stop=(i == N_WARM - 1),
        )

    def merged_load(eng, dst_tile, dram_ap, ntiles, tile0=0):
        # one DMA for `ntiles` 128x512 tiles: descriptor generation on the
        # issuing ring costs ~700ns per instruction, so per-tile dma_starts
        # serialize the head (~700ns each on the sequencer)
        df = dst_tile[:]
        dst = bass.AP(
            df.tensor, df.offset + tile0 * 512,
            [list(df.ap[0]), [512, ntiles], [1, 512]],
        )
        sf = dram_ap
        srco = sf.offset + tile0 * 65536
        s = bass.AP(sf.tensor, srco, [[512, 128], [65536, ntiles], [1, 512]])
        eng.dma_start(dst, s)

    with nc.named_scope("load_w"):
        # head-critical: per-tile DMAs so tile k lands as early as possible
        # (a merged DMA emits descriptors partition-major, so no tile
        # completes until the whole transfer ends); the rest are merged to
        # save ~700ns of sequencer descriptor-gen per dma_start
        for i in range(4):
            nc.sync.dma_start(wq_r[:, ts(i, 512)], wq[i * 128:(i + 1) * 128, :])
        xt0 = p_xt.tile([128, 4 * 512], f32r, tag="xt", name="xt0")
        for i in range(4):
            nc.scalar.dma_start(xt0[:, ts(i, 512)], x[0, i * 128:(i + 1) * 128, :])
        xload[0] = xt0
        for i in range(4):
            nc.sync.dma_start(wk_r[:, ts(i, 512)], wk[i * 128:(i + 1) * 128, :])
        # wv per-tile too: V(0)'s k-loop consumes tile k at ~18+0.9k us and a
        # merged load completes no tile until the whole 1MB lands
        for i in range(4):
            nc.sync.dma_start(wv_r[:, ts(i, 512)], wv[i * 128:(i + 1) * 128, :])

    zt_state = [None]
    pending = []

    def emit_z(b, vm, at):
        # ---- ZmT = lhsT(Vm natural) @ AT ----
        # PSUM->SBUF drain scatters straight into the WO-ready scrambled
        # layout: ZS free index = vblk*1024 + j*128 + (half*64+q) where the
        # Z column t = 8q + j and half = b%2.
        if b % 2 == 0:
            # zt stays f32r: the WO-ready scramble needs strided PSUM->SBUF
            # scatter writes, and 16-bit scattered writes run ~3x slower on
            # the DVE.  The WO matmul takes f32r stationary x bf16 moving.
            ztp = p_zt.tile([128, 2 * 4 * 512], f32r, tag="zt")
            zt_state[0] = ztp
        zt = zt_state[0]
        half = b % 2
        for vblk in range(4):
            pz = ps_mm.tile([128, 512], f32, tag="mm")
            for m in range(4):
                nc.tensor.matmul(
                    pz[:],
                    vm[:, m * 512 + vblk * 128: m * 512 + vblk * 128 + 128],
                    at[:, ts(m, 512)],
                    start=(m == 0), stop=(m == 3),
                )
            zf = zt[:]
            pf = pz[:]
            dst = bass.AP(
                zf.tensor, zf.offset + vblk * 1024 + half * 64,
                [list(zf.ap[0]), [1, 64], [128, 8]],
            )
            src = bass.AP(
                pf.tensor, pf.offset, [list(pf.ap[0]), [8, 64], [1, 8]]
            )
            nc.vector.tensor_copy(dst, src)

        # ---- WO stage for the (b-1, b) pair ----
        if b % 2 == 1:
            po = ps_wo.tile([128, 512], f32, tag="wops")
            for kt in range(32):
                # dblk-outer: the vblk-d scatter writes zt block d; reading
                # block 0 first keeps the PE off the blocks still in flight.
                # WO contraction row block for (j, dblk) is tile j*4 + dblk.
                # Exception: the first pair runs while the WO halves are
                # still streaming in tile-order, so keep tile-order there.
                if b == 1:
                    j, dblk = kt // 4, kt % 4
                else:
                    dblk, j = kt // 8, kt % 8
                rhs = wo_r[:, ts(j * 4 + dblk, 512)]
                lhs = zt[:, dblk * 1024 + j * 128: dblk * 1024 + (j + 1) * 128]
                nc.tensor.matmul(
                    po[:], lhs, rhs,
                    start=(kt == 0), stop=(kt == 31),
                )
            so = p_out.tile([128, 512], f32, tag="so")
            nc.scalar.activation(so[:], po[:], Copy)
            # rows of outp[b-1] and outp[b] are contiguous in DRAM: one DMA
            od = bass.AP(
                outp.tensor, outp.offset + (b - 1) * 64 * 512,
                [[512, 128], [1, 512]],
            )
            ring = nc.scalar if b == B - 1 else nc.sync
            ring.dma_start(od, so[:])

    def col(w, k, blk):
        return w[:, k * 512 + blk * 128: k * 512 + blk * 128 + 128]

    def load_wo_chunk(cb):
        # WO tiles are first read at the end of batch 2.  4-tile chunks: the
        # DMA system drains transfers in global issue order, so one 16-tile
        # merged DMA would block the sync sequencer for ~11us of descriptor
        # flow-control and hold every later transfer behind 4MB of FIFO.
        with nc.named_scope(f"load_wo{cb}"):
            merged_load(nc.sync, wo_r, wo, 4, tile0=4 * cb)

    for b in range(B):
        with nc.named_scope(f"batch{b}"):
            pass
            xt = xload.pop(b)

            # ---- QmT = WQ^T @ XT ----
            qt = p_q.tile([128, 4 * 512], f32r, tag="q")
            if b == 0:
                # k-outer: start the PE as soon as wq tile k / x tile k land
                pqs = [
                    ps_mm.tile([128, 512], f32, tag="mm", name=f"pq{i}")
                    for i in range(4)
                ]
                for k in range(4):
                    for dblk in range(4):
                        nc.tensor.matmul(
                            pqs[dblk][:],
                            col(wq_r, k, dblk), xt[:, ts(k, 512)],
                            start=(k == 0), stop=(k == 3),
                        )
                # alternate drain engines: scalar is idle until S(0), and
                # K(0)'s PSUM allocations wait on these (ps_mm recycling)
                for dblk in range(4):
                    if dblk % 2 == 0:
                        nc.vector.tensor_scalar_mul(
                            qt[:, ts(dblk, 512)], pqs[dblk][:], SCALE
                        )
                    else:
                        nc.scalar.activation(
                            qt[:, ts(dblk, 512)], pqs[dblk][:], Copy, scale=SCALE
                        )
            else:
                for dblk in range(4):
                    pq = ps_mm.tile([128, 512], f32, tag="mm")
                    for k in range(4):
                        nc.tensor.matmul(
                            pq[:], col(wq_r, k, dblk), xt[:, ts(k, 512)],
                            start=(k == 0), stop=(k == 3),
                        )
                    # fold the 1/sqrt(512) softmax scale into the drain copy
                    # (vector: the scalar queue still holds b-1's EXP/at ops,
                    # and K's PSUM banks wait on these drains)
                    nc.vector.tensor_scalar_mul(qt[:, ts(dblk, 512)], pq[:], SCALE)

            # ---- Km natural = XT^T-contract @ WK ----
            km = p_k.tile([128, 4 * 512], f32r, tag="k")
            if b == 0:
                pks = [
                    ps_mm.tile([128, 512], f32, tag="mm", name=f"pk{i}")
                    for i in range(4)
                ]
                for k in range(4):
                    for tblk in range(4):
                        nc.tensor.matmul(
                            pks[tblk][:],
                            col(xt, k, tblk), wk_r[:, ts(k, 512)],
                            start=(k == 0), stop=(k == 3),
                        )
                for tblk in range(4):
                    if tblk % 2 == 0:
                        nc.vector.tensor_copy(km[:, ts(tblk, 512)], pks[tblk][:])
                    else:
                        nc.scalar.activation(km[:, ts(tblk, 512)], pks[tblk][:], Copy)
            else:
                for tblk in range(4):
                    pk = ps_mm.tile([128, 512], f32, tag="mm")
                    for k in range(4):
                        nc.tensor.matmul(
                            pk[:], col(xt, k, tblk), wk_r[:, ts(k, 512)],
                            start=(k == 0), stop=(k == 3),
                        )
                    nc.vector.tensor_copy(km[:, ts(tblk, 512)], pk[:])

            # ---- Vm natural = XT^T-contract @ WV ----
            # before Z/S for b<15 so vm's PSUM drains clear the scalar queue
            # early (next batch's Q banks depend on them); for the last batch
            # V runs after S instead, covering the softmax latency before the
            # final Z+WO tail.
            def emit_v():
                vm = p_v.tile([128, 4 * 512], bf16, tag="v", name="vm")
                for tblk in range(4):
                    pv = ps_mm.tile([128, 512], f32, tag="mm", name="pv")
                    for k in range(4):
                        nc.tensor.matmul(
                            pv[:], col(xt, k, tblk), wv_r[:, ts(k, 512)],
                            start=(k == 0), stop=(k == 3),
                        )
                    nc.scalar.activation(vm[:, ts(tblk, 512)], pv[:], Copy)
                return vm

            if b == 0:
                load_x(1)
            vm = None if b == B - 1 else emit_v()
            if b == 0:
                # issue order = need order on the FIFO DMA stream: x2 jumps
                # ahead of WO (it gates batch 2's matmuls at ~50us; WO isn't
                # read before ~65us), then all eight WO chunks.
                load_x(2)
                for cb in range(8):
                    load_wo_chunk(cb)

            # deferred Z + WO of the previous batch: fills the PE while this
            # batch's qt/km drains land, and its matmuls hide S's wait
            if pending:
                emit_z(*pending.pop())

            # prefetch x two batches ahead AFTER the Z+WO stage: the DMA
            # writes then land during S/Q instead of fighting the WO
            # matmuls' zt/wo_r reads for SBUF bandwidth
            if b + 2 < B and b != 0:
                load_x(b + 2)

            # ---- ST = Km^T-contract @ QmT, softmax along free axis ----
            # A and the whole V/Z/WO chain run in bf16: the softmax weights
            # are near-one-hot so bf16 there costs ~0.4% relative error, and
            # bf16 operands halve WO DMA + SBUF read traffic
            at = p_at.tile([128, 4 * 512], bf16, tag="at")
            for sblk in range(4):
                pst = ps_st.tile([128, 512], f32, tag="st")
                for m in range(4):
                    nc.tensor.matmul(
                        pst[:], col(km, m, sblk), qt[:, ts(m, 512)],
                        start=(m == 0), stop=(m == 3),
                    )
                nmx = p_small.tile([128, 1], f32, tag="nmx")
                nc.vector.tensor_reduce(
                    nmx[:], pst[:], axis=mybir.AxisListType.X,
                    op=mybir.AluOpType.max, negate=True,
                )
                scr = p_scr.tile([128, 512], f32, tag="scr")
                sm = p_small.tile([128, 1], f32, tag="sm")
                nc.scalar.activation(
                    scr[:], pst[:], Exp, bias=nmx[:], scale=1.0, accum_out=sm[:],
                )
                rc = p_small.tile([128, 1], f32, tag="rc")
                nc.vector.reciprocal(rc[:], sm[:])
                if b == B - 1:
                    # tail: Z15 needs at+vm; at on vector lets the scalar
                    # queue reach the vm drains sooner
                    nc.vector.tensor_scalar_mul(at[:, ts(sblk, 512)], scr[:], rc[:])
                else:
                    nc.scalar.activation(
                        at[:, ts(sblk, 512)], scr[:], Copy, scale=rc[:]
                    )

            if vm is None:
                vm = emit_v()
            pending.append((b, vm, at))

    emit_z(*pending.pop())


def _build():
    import concourse.bass as bass  # noqa: F401
    import concourse.tile as tile
    from concourse import bacc, mybir

    nc = bacc.Bacc(
        "TRN2",
        target_bir_lowering=False,
        debug=False,
        enable_asserts=False,
        num_devices=N_CORES,
    )
    f32 = mybir.dt.float32
    f32r = mybir.dt.float32r
    aps = {
        "x": nc.dram_tensor("x", (B, E, T), f32r, kind="ExternalInput").ap(),
        "wq": nc.dram_tensor("wq", (E, E), f32r, kind="ExternalInput").ap(),
        "wk": nc.dram_tensor("wk", (E, E), f32r, kind="ExternalInput").ap(),
        "wv": nc.dram_tensor("wv", (E, E), f32r, kind="ExternalInput").ap(),
        "wo": nc.dram_tensor("wo", (H * E, E), f32r, kind="ExternalInput").ap(),
        "out": nc.dram_tensor("out", (B, 64, E), f32, kind="ExternalOutput").ap(),
    }
    from contextlib import ExitStack

    with tile.TileContext(nc) as tc, ExitStack() as ctx:
        _emit(ctx, nc, tc, tile, mybir, aps)
    nc.compile()
    return nc


def _get_nc():
    if "nc" not in _CACHE:
        _CACHE["nc"] = _build()
    return _CACHE["nc"]


def run(inputs, trace=False):
    from concourse.bass_utils import run_bass_kernel_spmd

    nc = _get_nc()
    x = np.asarray(inputs["x"], dtype=np.float32)
    xT = np.ascontiguousarray(x.transpose(0, 2, 1))
    WQ = np.asarray(inputs["WQ"], dtype=np.float32)
    WK = np.asarray(inputs["WK"], dtype=np.float32)
    WV = np.asarray(inputs["WV"], dtype=np.float32)
    WO = np.ascontiguousarray(np.asarray(inputs["WO"], dtype=np.float32))
    in_maps = [
        {
            "x": xT,
            "wq": np.ascontiguousarray(WQ[c]),
            "wk": np.ascontiguousarray(WK[c]),
            "wv": np.ascontiguousarray(WV[c]),
            "wo": WO,
        }
        for c in range(N_CORES)
    ]
    res = run_bass_kernel_spmd(
        nc, in_maps, core_ids=list(range(N_CORES)), trace=trace
    )
    out = np.empty((B, T, E), dtype=np.float32)
    for c in range(N_CORES):
        out[:, 64 * c:64 * (c + 1), :] = res.results[c]["out"]
    return out, res


def kernel(**inputs):
    out, _ = run(inputs, trace=False)
    return out

